# revision 2
# baseline (speedup 1.0000x reference)
"""GroupGAT kernel for Trainium2 (Bass/Tile), 8-core data-parallel.

Math restructure (attention weights commute with @W):
    H = h @ W;  e[b,n] = lrelu(H_self.a1 + H[b,n].a2)
              = lrelu(h_self.(W a1) + h[b,n].(W a2))       <- dots in h-space
    ws = softmax(e) @ H = (softmax(e) @ h) @ W             <- weighted sum in h-space
    out = elu(H_ally[:,0] + ws_ally + ws_opp)
        = elu((h_self + hw_ally) @ W_ally + hw_opp @ W_opp)

Engine mapping (v2 — everything heavy on PE, bf16 data):
  - h arrives bf16; an SBUF->SBUF xbar DMA transpose produces hT [d, n, b].
  - dots: per-node matmul lhsT=hT[:,n,:] ([d,b]) x rhs=vcat ([d,4]) -> e
    slices [b, 4] in PSUM (contract over d). 41 MMs, vcat columns are
    (W_ally a1 | W_ally a2 | W_opp a1 | W_opp a2).
  - softmax pieces on DVE/ACT (tiny [P,42] ops), normalized weights w01
    (with +1 on the ally self column) cast to bf16.
  - diag(w01[:,n]) matrices materialized in one [P,128,42] buffer: ally
    half on DVE (identity-mask * broadcast multiply, 2x mode), opp half
    on the otherwise-idle GPSIMD via affine_select (iota b-p == 0).
  - weighted sums: per-node matmul lhsT=h[:,n,:] x rhs=diag[:,:,n] accumulated
    in PSUM -> hwT [d, b] directly transposed for the final matmuls.
  - final: out[b,c] = lhsT=hwT_x x rhs=W halves, accumulated; elu; DMA out.
"""

import numpy as np
import ml_dtypes

import concourse.bass as bass
import concourse.bacc as bacc
import concourse.mybir as mybir
from concourse import tile
from concourse.bass_utils import run_bass_kernel_spmd

N_CORES = 8
B = 16384
NN = 41  # num nodes
NA = 20  # num_ally
NO = 20  # num_opp
D = 128
B_SHARD = B // N_CORES
P = 128
NEG_INF = -1e9

F32 = mybir.dt.float32
BF16 = mybir.dt.bfloat16
AL = mybir.AluOpType
AF = mybir.ActivationFunctionType
BF16_NP = ml_dtypes.bfloat16


def build_nc(b_shard=B_SHARD):
    n_tiles = b_shard // P
    nc = bacc.Bacc("TRN2", target_bir_lowering=False, debug=False)

    h_d = nc.dram_tensor("h", [b_shard, NN, D], BF16, kind="ExternalInput").ap()
    mneg_d = nc.dram_tensor("mneg", [b_shard, 42], F32, kind="ExternalInput").ap()
    vcat_d = nc.dram_tensor("vcat", [D, 4], BF16, kind="ExternalInput").ap()
    wcat_d = nc.dram_tensor("wcat", [D, 2 * D], BF16, kind="ExternalInput").ap()
    maskf_d = nc.dram_tensor("maskf", [P, D, 42], BF16, kind="ExternalInput").ap()
    out_d = nc.dram_tensor("out", [b_shard, D], F32, kind="ExternalOutput").ap()

    with tile.TileContext(nc) as tc:
        with (
            tc.tile_pool(name="const", bufs=1) as cpool,
            tc.tile_pool(name="hin", bufs=3) as hpool,
            tc.tile_pool(name="ht", bufs=3) as htpool,
            tc.tile_pool(name="diag", bufs=2) as dpool,
            tc.tile_pool(name="small", bufs=3) as spool,
            tc.tile_pool(name="work", bufs=3) as wpool,
            tc.tile_pool(name="psum", bufs=2, space=bass.MemorySpace.PSUM) as ppool,
        ):
            vcat = cpool.tile([D, 4], BF16)
            wcat = cpool.tile([D, 2 * D], BF16)
            maskf = cpool.tile([P, D, 42], BF16)
            nc.sync.dma_start(vcat[:], vcat_d[:])
            nc.sync.dma_start(wcat[:], wcat_d[:])
            nc.sync.dma_start(maskf[:], maskf_d[:])

            for it in range(n_tiles):
                b0 = it * P
                h_t = hpool.tile([P, NN, D], BF16)
                mneg_t = spool.tile([P, 42], F32, tag="mneg")
                nc.sync.dma_start(h_t[:], h_d[b0 : b0 + P])
                nc.sync.dma_start(mneg_t[:], mneg_d[b0 : b0 + P])

                # hT[d, n, b] = h[b, n, d] via xbar transpose (SBUF->SBUF)
                hT = htpool.tile([P, NN, D], BF16)
                nc.scalar.dma_start_transpose(hT[:], h_t[:])

                # --- dots on PE: e_ps[b, n, g] = sum_d hT[d,n,b] * vcat[d,g]
                e_ps = ppool.tile([P, NN, 4], F32, tag="eps")
                for n in range(NN):
                    nc.tensor.matmul(
                        e_ps[:, n, :], hT[:, n, :], vcat[:], start=True, stop=True
                    )

                # --- e assembly: e = (dots_v2 + s1) + mneg; lrelu
                e_pre = spool.tile([P, 42], F32, tag="epre")
                nc.vector.scalar_tensor_tensor(
                    e_pre[:, 0:21], e_ps[:, 0:21, 1], e_ps[:, 0:1, 0],
                    mneg_t[:, 0:21], AL.add, AL.add,
                )
                nc.vector.tensor_scalar_add(
                    e_pre[:, 21:22], e_ps[:, 0:1, 3], e_ps[:, 0:1, 2]
                )
                nc.vector.scalar_tensor_tensor(
                    e_pre[:, 22:42], e_ps[:, 21:41, 3], e_ps[:, 0:1, 2],
                    mneg_t[:, 22:42], AL.add, AL.add,
                )
                nc.vector.scalar_tensor_tensor(
                    e_pre[:], e_pre[:], 0.2, e_pre[:], AL.mult, AL.max
                )

                # --- exp + denominators + normalized weights (bf16)
                expe = spool.tile([P, 42], F32, tag="expe")
                den = spool.tile([P, 2], F32, tag="den")
                rec = spool.tile([P, 2], F32, tag="rec")
                nc.scalar.activation(
                    expe[:, 0:21], e_pre[:, 0:21], AF.Exp, accum_out=den[:, 0:1]
                )
                nc.scalar.activation(
                    expe[:, 21:42], e_pre[:, 21:42], AF.Exp, accum_out=den[:, 1:2]
                )
                nc.vector.reciprocal(rec[:], den[:])
                w01 = spool.tile([P, 42], BF16, tag="w01")
                nc.vector.tensor_scalar_mul(w01[:, 0:21], expe[:, 0:21], rec[:, 0:1])
                nc.vector.tensor_scalar_mul(w01[:, 21:42], expe[:, 21:42], rec[:, 1:2])
                nc.vector.tensor_scalar_add(w01[:, 0:1], w01[:, 0:1], 1.0)

                # --- diag materialization: diag[p, b, n] = w01[p, n] * (b == p)
                diag = dpool.tile([P, D, 42], BF16)
                nc.vector.tensor_mul(
                    diag[:, :, 0:21],
                    maskf[:, :, 0:21],
                    w01[:, None, 0:21].broadcast_to([P, D, 21]),
                )
                nc.gpsimd.affine_select(
                    diag[:, :, 21:42],
                    w01[:, None, 21:42].broadcast_to([P, D, 21]),
                    pattern=[[1, D], [0, 21]],
                    compare_op=AL.is_equal,
                    fill=0.0,
                    base=0,
                    channel_multiplier=-1,
                )

                # --- weighted sums on PE: hwT[d, b] = sum_n h[b,n,d]*w01[b,n]
                hwT_a = ppool.tile([P, D], F32, tag="hwa")
                hwT_o = ppool.tile([P, D], F32, tag="hwo")
                for k in range(21):
                    nc.tensor.matmul(
                        hwT_a[:], h_t[:, k, :], diag[:, :, k],
                        start=(k == 0), stop=(k == 20),
                    )
                for k in range(21):
                    src = 0 if k == 0 else NA + k
                    nc.tensor.matmul(
                        hwT_o[:], h_t[:, src, :], diag[:, :, 21 + k],
                        start=(k == 0), stop=(k == 20),
                    )

                xT_a = wpool.tile([P, D], BF16, tag="xta")
                xT_o = wpool.tile([P, D], BF16, tag="xto")
                nc.scalar.copy(xT_a[:], hwT_a[:])
                nc.scalar.copy(xT_o[:], hwT_o[:])

                # --- out = elu(xT_a.T @ W_ally + xT_o.T @ W_opp)
                out_ps = ppool.tile([P, D], F32, tag="ops")
                nc.tensor.matmul(out_ps[:], xT_a[:], wcat[:, 0:D], start=True, stop=False)
                nc.tensor.matmul(out_ps[:], xT_o[:], wcat[:, D : 2 * D], start=False, stop=True)

                # elu(x) = max(x, exp(min(x,0)) - 1)
                t1 = wpool.tile([P, D], F32, tag="t1")
                out_t = wpool.tile([P, D], F32, tag="outt")
                nc.vector.tensor_scalar_min(t1[:], out_ps[:], 0.0)
                nc.scalar.activation(t1[:], t1[:], AF.Exp)
                nc.vector.scalar_tensor_tensor(
                    out_t[:], t1[:], -1.0, out_ps[:], AL.add, AL.max
                )
                nc.sync.dma_start(out_d[b0 : b0 + P], out_t[:])

    nc.compile()
    return nc


_NC_CACHE = {}


def _get_nc(b_shard):
    if b_shard not in _NC_CACHE:
        _NC_CACHE[b_shard] = build_nc(b_shard)
    return _NC_CACHE[b_shard]


def _host_precompute(W_ally, W_opp, a_ally, a_opp, mask):
    v1a = W_ally @ a_ally[:D, 0]
    v2a = W_ally @ a_ally[D:, 0]
    v1o = W_opp @ a_opp[:D, 0]
    v2o = W_opp @ a_opp[D:, 0]
    vcat = np.ascontiguousarray(
        np.stack([v1a, v2a, v1o, v2o], axis=1).astype(BF16_NP)
    )
    wcat = np.ascontiguousarray(
        np.concatenate([W_ally, W_opp], axis=1).astype(BF16_NP)
    )
    eye = (np.arange(P)[:, None] == np.arange(D)[None, :]).astype(BF16_NP)
    maskf = np.ascontiguousarray(np.repeat(eye[:, :, None], 42, axis=2))
    b = mask.shape[0]
    mneg = np.zeros((b, 42), np.float32)
    mneg[:, 1:21] = np.where(mask[:, 1 : 1 + NA], NEG_INF, 0.0)
    mneg[:, 22:42] = np.where(mask[:, 1 + NA :], NEG_INF, 0.0)
    return vcat, wcat, maskf, mneg


def kernel(h, W_ally, W_opp, a_ally, a_opp, mask, num_ally, num_opp):
    assert int(num_ally) == NA and int(num_opp) == NO
    h = np.asarray(h, dtype=np.float32)
    mask = np.asarray(mask)
    W_ally = np.asarray(W_ally, dtype=np.float32)
    W_opp = np.asarray(W_opp, dtype=np.float32)
    a_ally = np.asarray(a_ally, dtype=np.float32)
    a_opp = np.asarray(a_opp, dtype=np.float32)

    vcat, wcat, maskf, mneg = _host_precompute(W_ally, W_opp, a_ally, a_opp, mask)
    h_bf = np.ascontiguousarray(h.astype(BF16_NP))

    nc = _get_nc(B_SHARD)
    in_maps = []
    for c in range(N_CORES):
        s = slice(c * B_SHARD, (c + 1) * B_SHARD)
        in_maps.append(
            {
                "h": h_bf[s],
                "mneg": np.ascontiguousarray(mneg[s]),
                "vcat": vcat,
                "wcat": wcat,
                "maskf": maskf,
            }
        )
    res = run_bass_kernel_spmd(nc, in_maps, core_ids=list(range(N_CORES)))
    global LAST_RESULTS
    LAST_RESULTS = res
    return np.concatenate([res.results[c]["out"] for c in range(N_CORES)], axis=0)


LAST_RESULTS = None


# revision 9
# speedup vs baseline: 1.0692x; 1.0692x over previous
"""GroupGAT kernel for Trainium2 (Bass/Tile), 8-core data-parallel.

Math restructure (attention weights commute with @W):
    e[b,n] = lrelu(h_self.(W a1) + h[b,n].(W a2))    <- dots in h-space
    out = elu((h_self + hw_ally) @ W_ally + hw_opp @ W_opp),
    hw_x[b,:] = sum_n w_x[b,n] h[b,n,:]              <- weighted sums in h-space

Measured-fact-driven engine mapping (v2.7):
  - h loaded as ONE flat 2D DMA per tile (avoids descriptor shattering).
  - hT [d, n, b] via one full-tile xbar DMA transpose (offset-0 APs only --
    sliced xbar destinations produce wrong results on HW), ring alternating
    between SP and ACT per tile.
  - dots: 41 PE matmuls lhsT=hT[:,n,:] x rhs=vcat[d,4] -> e_ps[b, n, 4]
    (alternating-stationary pairs pipeline at ~25ns for N=4).
  - tensor_scalar with AP scalars is pathological (~1.2us); per-partition
    scaling uses scalar_tensor_tensor or ACT instead.
  - diag(w01) [P, d, 44] (n-innermost, padded even for DVE 2x mode) built by
    ONE TT multiply against a constant identity mask (DVE half) and
    affine_select (GPSIMD half).
  - weighted sums: per-node matmul lhsT=h_n x rhs=diag[:,:,j] accumulating
    hwT [d, b] in PSUM -- already transposed for the finals.
  - finals: lhsT=xT (ACT-copied bf16) x rhs=W halves -> out[b,c]; elu; DMA.
"""

import numpy as np
import ml_dtypes

import concourse.bass as bass
import concourse.bacc as bacc
import concourse.mybir as mybir
from concourse import tile
from concourse.bass_utils import run_bass_kernel_spmd

N_CORES = 8
B = 16384
NN = 41
NA = 20
NO = 20
D = 128
B_SHARD = B // N_CORES
P = 128
NEG_INF = -1e9
NJ = 44  # diag slots padded even: 0..20 ally, 21..41 opp, 42..43 zero

F32 = mybir.dt.float32
BF16 = mybir.dt.bfloat16
AL = mybir.AluOpType
AF = mybir.ActivationFunctionType
BF16_NP = ml_dtypes.bfloat16

DVE_DIAG = 28  # diag cols 0..27 on DVE; 28..43 on GPSIMD


def _h_node_of_slot(j):
    if j <= 20:
        return j
    if j == 21:
        return 0
    return j - 1  # 22..41 -> h nodes 21..40


def build_nc(b_shard=B_SHARD):
    n_tiles = b_shard // P
    nc = bacc.Bacc("TRN2", target_bir_lowering=False, debug=False)

    h_d = nc.dram_tensor("h", [b_shard, NN * D], BF16, kind="ExternalInput").ap()
    mneg_d = nc.dram_tensor("mneg", [b_shard, 42], F32, kind="ExternalInput").ap()
    vcat_d = nc.dram_tensor("vcat", [D, 4], BF16, kind="ExternalInput").ap()
    wcat_d = nc.dram_tensor("wcat", [D, 2 * D], BF16, kind="ExternalInput").ap()
    maskf_d = nc.dram_tensor("maskf", [P, D * NJ], BF16, kind="ExternalInput").ap()
    ones_d = nc.dram_tensor("ones", [P, 42], F32, kind="ExternalInput").ap()
    out_d = nc.dram_tensor("out", [b_shard, D], F32, kind="ExternalOutput").ap()

    with tile.TileContext(nc) as tc:
        with (
            tc.tile_pool(name="const", bufs=1) as cpool,
            tc.tile_pool(name="hin", bufs=3) as hpool,
            tc.tile_pool(name="ht", bufs=3) as htpool,
            tc.tile_pool(name="diag", bufs=2) as dpool,
            tc.tile_pool(name="small", bufs=3) as spool,
            tc.tile_pool(name="work", bufs=3) as wpool,
            tc.tile_pool(name="psum", bufs=2, space=bass.MemorySpace.PSUM) as ppool,
        ):
            vcat = cpool.tile([D, 4], BF16)
            wcat = cpool.tile([D, 2 * D], BF16)
            maskf = cpool.tile([P, D, NJ], BF16)
            ones42 = cpool.tile([P, 42], F32)
            nc.sync.dma_start(vcat[:], vcat_d[:])
            nc.sync.dma_start(wcat[:], wcat_d[:])
            nc.sync.dma_start(maskf[:], maskf_d[:])
            nc.sync.dma_start(ones42[:], ones_d[:])

            def phase_a(it):
                b0 = it * P
                h_t = hpool.tile([P, NN * D], BF16)
                mneg_t = spool.tile([P, 42], F32, tag="mneg")
                nc.sync.dma_start(h_t[:], h_d[b0 : b0 + P])
                nc.sync.dma_start(mneg_t[:], mneg_d[b0 : b0 + P])

                def hnode(n):
                    return h_t[:, n * D : (n + 1) * D]

                # hT[d, n, b] = h[b, n, d]; full-tile xbar, alternate rings
                hT = htpool.tile([P, NN, D], BF16)
                nc.scalar.dma_start_transpose(hT[:], h_t[:])

                # --- dots on PE: e_ps[b, n, g] = sum_d hT[d,n,b] * vcat[d,g]
                e_ps = ppool.tile([P, NN, 4], F32, tag="eps")
                for n in range(NN):
                    nc.tensor.matmul(
                        e_ps[:, n, :], hT[:, n, :], vcat[:], start=True, stop=True
                    )

                # --- e assembly (cols: 0..20 ally, 21..41 opp) ---
                e_pre = spool.tile([P, 42], F32, tag="epre")
                s1a = e_ps[:, 0:1, 0]
                s1o = e_ps[:, 0:1, 2]
                nc.vector.scalar_tensor_tensor(
                    e_pre[:, 0:21], e_ps[:, 0:21, 1], s1a,
                    mneg_t[:, 0:21], AL.add, AL.add,
                )
                nc.vector.tensor_scalar_add(e_pre[:, 21:22], e_ps[:, 0:1, 3], s1o)
                nc.vector.scalar_tensor_tensor(
                    e_pre[:, 22:42], e_ps[:, 21:NN, 3], s1o,
                    mneg_t[:, 22:42], AL.add, AL.add,
                )
                nc.vector.scalar_tensor_tensor(
                    e_pre[:], e_pre[:], 0.2, e_pre[:], AL.mult, AL.max
                )

                # --- softmax weights (w01 bf16 [P, 44], pad cols zeroed) ---
                expe = spool.tile([P, 42], F32, tag="expe")
                den = spool.tile([P, 2], F32, tag="den")
                rec = spool.tile([P, 2], F32, tag="rec")
                nc.scalar.activation(
                    expe[:, 0:21], e_pre[:, 0:21], AF.Exp, accum_out=den[:, 0:1]
                )
                nc.scalar.activation(
                    expe[:, 21:42], e_pre[:, 21:42], AF.Exp, accum_out=den[:, 1:2]
                )
                nc.vector.reciprocal(rec[:], den[:])
                w01 = spool.tile([P, NJ], BF16, tag="w01")
                nc.vector.scalar_tensor_tensor(
                    w01[:, 0:21], expe[:, 0:21], rec[:, 0:1],
                    ones42[:, 0:21], AL.mult, AL.mult,
                )
                nc.vector.scalar_tensor_tensor(
                    w01[:, 21:42], expe[:, 21:42], rec[:, 1:2],
                    ones42[:, 21:42], AL.mult, AL.mult,
                )
                nc.vector.tensor_scalar_add(w01[:, 0:1], w01[:, 0:1], 1.0)
                nc.vector.memset(w01[:, 42:44], 0.0)

                # --- diag materialization: diag[p, d, j] = w01[p, j]*(d == p)
                diag = dpool.tile([P, D, NJ], BF16)
                nc.vector.tensor_mul(
                    diag[:, :, 0:DVE_DIAG],
                    maskf[:, :, 0:DVE_DIAG],
                    w01[:, None, 0:DVE_DIAG].broadcast_to([P, D, DVE_DIAG]),
                )
                nc.gpsimd.affine_select(
                    diag[:, :, DVE_DIAG:NJ],
                    w01[:, None, DVE_DIAG:NJ].broadcast_to([P, D, NJ - DVE_DIAG]),
                    pattern=[[1, D], [0, NJ - DVE_DIAG]],
                    compare_op=AL.is_equal,
                    fill=0.0,
                    base=0,
                    channel_multiplier=-1,
                )

                return h_t, diag

            def phase_b(it, h_t, diag):
                b0 = it * P

                def hnode(n):
                    return h_t[:, n * D : (n + 1) * D]

                # --- weighted sums on PE: hwT[d, b] += h_n[b,d]*w01[b,j]
                hwT_a = ppool.tile([P, D], F32, tag="hwa")
                hwT_o = ppool.tile([P, D], F32, tag="hwo")
                for k in range(21):
                    nc.tensor.matmul(
                        hwT_a[:], hnode(k), diag[:, :, k],
                        start=(k == 0), stop=(k == 20),
                    )
                for k in range(21):
                    nc.tensor.matmul(
                        hwT_o[:], hnode(_h_node_of_slot(21 + k)), diag[:, :, 21 + k],
                        start=(k == 0), stop=(k == 20),
                    )

                xT_a = wpool.tile([P, D], BF16, tag="xta")
                xT_o = wpool.tile([P, D], BF16, tag="xto")
                nc.scalar.copy(xT_a[:], hwT_a[:])
                nc.scalar.copy(xT_o[:], hwT_o[:])

                # --- out = elu(xT_a.T @ W_ally + xT_o.T @ W_opp)
                out_ps = ppool.tile([P, D], F32, tag="ops")
                nc.tensor.matmul(out_ps[:], xT_a[:], wcat[:, 0:D], start=True, stop=False)
                nc.tensor.matmul(out_ps[:], xT_o[:], wcat[:, D : 2 * D], start=False, stop=True)

                # elu(x) = max(x, exp(min(x,0)) - 1)
                t1 = wpool.tile([P, D], F32, tag="t1")
                out_t = wpool.tile([P, D], F32, tag="outt")
                nc.vector.tensor_scalar_min(t1[:], out_ps[:], 0.0)
                nc.scalar.activation(t1[:], t1[:], AF.Exp)
                nc.vector.scalar_tensor_tensor(
                    out_t[:], t1[:], -1.0, out_ps[:], AL.add, AL.max
                )
                nc.sync.dma_start(out_d[b0 : b0 + P], out_t[:])

            prev = None
            for it in range(n_tiles):
                state = phase_a(it)
                if prev is not None:
                    phase_b(it - 1, *prev)
                prev = state
            phase_b(n_tiles - 1, *prev)

    nc.compile()
    return nc


_NC_CACHE = {}


def _get_nc(b_shard):
    if b_shard not in _NC_CACHE:
        _NC_CACHE[b_shard] = build_nc(b_shard)
    return _NC_CACHE[b_shard]


def _host_precompute(W_ally, W_opp, a_ally, a_opp, mask):
    v1a = W_ally @ a_ally[:D, 0]
    v2a = W_ally @ a_ally[D:, 0]
    v1o = W_opp @ a_opp[:D, 0]
    v2o = W_opp @ a_opp[D:, 0]
    vcat = np.ascontiguousarray(np.stack([v1a, v2a, v1o, v2o], axis=1).astype(BF16_NP))
    wcat = np.ascontiguousarray(np.concatenate([W_ally, W_opp], axis=1).astype(BF16_NP))
    eye = (np.arange(P)[:, None] == np.arange(D)[None, :]).astype(BF16_NP)
    maskf = np.ascontiguousarray(np.repeat(eye[:, :, None], NJ, axis=2).reshape(P, D * NJ))
    b = mask.shape[0]
    mneg = np.zeros((b, 42), np.float32)
    mneg[:, 1:21] = np.where(mask[:, 1 : 1 + NA], NEG_INF, 0.0)
    mneg[:, 22:42] = np.where(mask[:, 1 + NA :], NEG_INF, 0.0)
    ones = np.ones((P, 42), np.float32)
    return vcat, wcat, maskf, ones, mneg


def kernel(h, W_ally, W_opp, a_ally, a_opp, mask, num_ally, num_opp):
    assert int(num_ally) == NA and int(num_opp) == NO
    h = np.asarray(h, dtype=np.float32)
    mask = np.asarray(mask)
    W_ally = np.asarray(W_ally, dtype=np.float32)
    W_opp = np.asarray(W_opp, dtype=np.float32)
    a_ally = np.asarray(a_ally, dtype=np.float32)
    a_opp = np.asarray(a_opp, dtype=np.float32)

    vcat, wcat, maskf, ones, mneg = _host_precompute(W_ally, W_opp, a_ally, a_opp, mask)
    bfull = h.shape[0]
    h_bf = np.ascontiguousarray(h.reshape(bfull, NN * D).astype(BF16_NP))

    nc = _get_nc(B_SHARD)
    in_maps = []
    for c in range(N_CORES):
        s = slice(c * B_SHARD, (c + 1) * B_SHARD)
        in_maps.append(
            {
                "h": h_bf[s],
                "mneg": np.ascontiguousarray(mneg[s]),
                "vcat": vcat,
                "wcat": wcat,
                "maskf": maskf,
                "ones": ones,
            }
        )
    res = run_bass_kernel_spmd(nc, in_maps, core_ids=list(range(N_CORES)))
    global LAST_RESULTS
    LAST_RESULTS = res
    return np.concatenate([res.results[c]["out"] for c in range(N_CORES)], axis=0)


LAST_RESULTS = None


# revision 13
# speedup vs baseline: 1.2647x; 1.1829x over previous
"""GroupGAT kernel for Trainium2 (Bass/Tile), 8-core data-parallel.

Math restructure (attention weights commute with @W):
    e[b,n] = lrelu(h_self.(W a1) + h[b,n].(W a2))    <- dots in h-space
    out = elu((h_self + hw_ally) @ W_ally + hw_opp @ W_opp),
    hw_x[b,:] = sum_n w_x[b,n] h[b,n,:]              <- weighted sums in h-space

Measured-fact-driven engine mapping (v2.9):
  - h loaded as ONE flat 2D DMA per tile; hT via full-tile xbar transpose
    (offset-0 APs), ring alternating SP/ACT per tile.
  - dots: 41 PE matmuls lhsT=hT[:,n,:] x rhs=vcat[d,4] -> e_ps[b,n,4]
    (measured 26ns cadence).
  - weighted sums: the diag(w01) matrix is nonzero only in its four 32x32
    diagonal blocks. diagblk [P, 44, 32] holds those blocks (c innermost,
    contiguous for LDWEIGHTS); built by ONE GPSIMD TT multiply against a
    constant block-mask. Per node, 4 concurrent tile-positioned sub-matmuls
    (lhsT=diagblk block, rhs=h_n partition-slice, both contiguous) accumulate
    hw[b, d] in PSUM.
  - finals: hw -> DVE copy bf16 -> small xbar transpose -> lhsT=xT x W.
  - 2-deep software pipeline so PE never waits on the softmax/diag chain.
"""

import numpy as np
import ml_dtypes

import concourse.bass as bass
import concourse.bacc as bacc
import concourse.mybir as mybir
from concourse import tile
from concourse.bass_utils import run_bass_kernel_spmd

N_CORES = 8
B = 16384
NN = 41
NA = 20
NO = 20
D = 128
B_SHARD = B // N_CORES
P = 128
NEG_INF = -1e9
NJ = 44  # diag slots padded even: 0..20 ally, 21..41 opp, 42..43 zero

F32 = mybir.dt.float32
BF16 = mybir.dt.bfloat16
AL = mybir.AluOpType
AF = mybir.ActivationFunctionType
BF16_NP = ml_dtypes.bfloat16


def _h_node_of_slot(j):
    if j <= 20:
        return j
    if j == 21:
        return 0
    return j - 1  # 22..41 -> h nodes 21..40


def build_nc(b_shard=B_SHARD):
    n_tiles = b_shard // P
    nc = bacc.Bacc("TRN2", target_bir_lowering=False, debug=False)

    h_d = nc.dram_tensor("h", [b_shard, NN * D], BF16, kind="ExternalInput").ap()
    mneg_d = nc.dram_tensor("mneg", [b_shard, 42], F32, kind="ExternalInput").ap()
    vcat_d = nc.dram_tensor("vcat", [D, 4], BF16, kind="ExternalInput").ap()
    wcat_d = nc.dram_tensor("wcat", [D, 2 * D], BF16, kind="ExternalInput").ap()
    mask3_d = nc.dram_tensor("mask3", [P, NJ * 32], BF16, kind="ExternalInput").ap()
    ones_d = nc.dram_tensor("ones", [P, 42], F32, kind="ExternalInput").ap()
    ident_d = nc.dram_tensor("ident", [P, P], BF16, kind="ExternalInput").ap()
    out_d = nc.dram_tensor("out", [b_shard, D], F32, kind="ExternalOutput").ap()

    with tile.TileContext(nc) as tc:
        with (
            tc.tile_pool(name="const", bufs=1) as cpool,
            tc.tile_pool(name="hin", bufs=4) as hpool,
            tc.tile_pool(name="ht", bufs=3) as htpool,
            tc.tile_pool(name="diag", bufs=3) as dpool,
            tc.tile_pool(name="small", bufs=4) as spool,
            tc.tile_pool(name="work", bufs=3) as wpool,
            tc.tile_pool(name="psum", bufs=2, space=bass.MemorySpace.PSUM) as ppool,
            tc.tile_pool(name="psum1", bufs=1, space=bass.MemorySpace.PSUM) as ppool1,
        ):
            vcat = cpool.tile([D, 4], BF16)
            wcat = cpool.tile([D, 2 * D], BF16)
            mask3 = cpool.tile([P, NJ, 32], BF16)
            ones42 = cpool.tile([P, 42], F32)
            ident = cpool.tile([P, P], BF16)
            nc.sync.dma_start(vcat[:], vcat_d[:])
            nc.sync.dma_start(wcat[:], wcat_d[:])
            nc.sync.dma_start(mask3[:], mask3_d[:])
            nc.sync.dma_start(ones42[:], ones_d[:])
            nc.sync.dma_start(ident[:], ident_d[:])

            def phase_a(it):
                b0 = it * P
                h_t = hpool.tile([P, NN * D], BF16)
                mneg_t = spool.tile([P, 42], F32, tag="mneg")
                nc.sync.dma_start(h_t[:], h_d[b0 : b0 + P])
                nc.sync.dma_start(mneg_t[:], mneg_d[b0 : b0 + P])

                # hT[d, n, b] = h[b, n, d]; full-tile xbar, alternate rings
                hT = htpool.tile([P, NN, D], BF16)
                nc.scalar.dma_start_transpose(hT[:], h_t[:])

                # --- dots on PE: e_ps[b, n, g] = sum_d hT[d,n,b] * vcat[d,g]
                e_ps = ppool1.tile([P, NN, 4], F32, tag="eps")
                for n in range(NN):
                    nc.tensor.matmul(
                        e_ps[:, n, :], hT[:, n, :], vcat[:], start=True, stop=True
                    )

                # --- e assembly (cols: 0..20 ally, 21..41 opp) ---
                e_pre = spool.tile([P, 42], F32, tag="epre")
                s1a = e_ps[:, 0:1, 0]
                s1o = e_ps[:, 0:1, 2]
                nc.vector.scalar_tensor_tensor(
                    e_pre[:, 0:21], e_ps[:, 0:21, 1], s1a,
                    mneg_t[:, 0:21], AL.add, AL.add,
                )
                nc.vector.tensor_scalar_add(e_pre[:, 21:22], e_ps[:, 0:1, 3], s1o)
                nc.vector.scalar_tensor_tensor(
                    e_pre[:, 22:42], e_ps[:, 21:NN, 3], s1o,
                    mneg_t[:, 22:42], AL.add, AL.add,
                )
                nc.vector.scalar_tensor_tensor(
                    e_pre[:], e_pre[:], 0.2, e_pre[:], AL.mult, AL.max
                )

                # --- softmax weights (w01 bf16 [P, 44], pad cols zeroed) ---
                expe = spool.tile([P, 42], F32, tag="expe")
                den = spool.tile([P, 2], F32, tag="den")
                rec = spool.tile([P, 2], F32, tag="rec")
                nc.scalar.activation(
                    expe[:, 0:21], e_pre[:, 0:21], AF.Exp, accum_out=den[:, 0:1]
                )
                nc.scalar.activation(
                    expe[:, 21:42], e_pre[:, 21:42], AF.Exp, accum_out=den[:, 1:2]
                )
                nc.vector.reciprocal(rec[:], den[:])
                w01 = spool.tile([P, NJ], BF16, tag="w01")
                nc.vector.scalar_tensor_tensor(
                    w01[:, 0:21], expe[:, 0:21], rec[:, 0:1],
                    ones42[:, 0:21], AL.mult, AL.mult,
                )
                nc.vector.scalar_tensor_tensor(
                    w01[:, 21:42], expe[:, 21:42], rec[:, 1:2],
                    ones42[:, 21:42], AL.mult, AL.mult,
                )
                nc.vector.tensor_scalar_add(w01[:, 0:1], w01[:, 0:1], 1.0)
                nc.vector.memset(w01[:, 42:44], 0.0)

                # --- block-diag: diagblk[p, j, c] = w01[p, j]*(c == p%32)
                diagblk = dpool.tile([P, NJ, 32], BF16)
                nc.gpsimd.tensor_mul(
                    diagblk[:],
                    mask3[:],
                    w01[:, :, None].broadcast_to([P, NJ, 32]),
                )
                return h_t, diagblk

            def phase_b(it, h_t, diagblk):
                b0 = it * P

                def hnode(n):
                    return h_t[:, n * D : (n + 1) * D]

                # --- weighted sums on PE, 4 concurrent 32x32 blocks/node ---
                hwps = ppool.tile([P, 2, D], F32, tag="hw")
                hw_a = hwps[:, 0, :]
                hw_o = hwps[:, 1, :]
                for grp, hw in ((0, hw_a), (1, hw_o)):
                    for k in range(21):
                        j = grp * 21 + k
                        hn = hnode(_h_node_of_slot(j))
                        for i in range(4):
                            r = slice(32 * i, 32 * i + 32)
                            nc.tensor.matmul(
                                hw[r], diagblk[r, j, :], hn[r, :],
                                start=(k == 0), stop=(k == 20),
                                tile_position=(32 * i, 32 * i),
                                skip_group_check=True,
                            )

                x_a = wpool.tile([P, D], BF16, tag="xa")
                x_o = wpool.tile([P, D], BF16, tag="xo")
                nc.vector.tensor_copy(x_a[:], hw_a)
                nc.vector.tensor_copy(x_o[:], hw_o)
                trp = ppool1.tile([P, 2, D], BF16, tag="trp")
                trp_a = trp[:, 0, :]
                trp_o = trp[:, 1, :]
                nc.tensor.transpose(trp_a, x_a[:], ident[:])
                nc.tensor.transpose(trp_o, x_o[:], ident[:])
                xT_a = wpool.tile([P, D], BF16, tag="xta")
                xT_o = wpool.tile([P, D], BF16, tag="xto")
                nc.vector.tensor_copy(xT_a[:], trp_a)
                nc.vector.tensor_copy(xT_o[:], trp_o)

                # --- out = elu(xT_a.T @ W_ally + xT_o.T @ W_opp)
                out_ps = ppool.tile([P, D], F32, tag="ops")
                nc.tensor.matmul(out_ps[:], xT_a[:], wcat[:, 0:D], start=True, stop=False)
                nc.tensor.matmul(out_ps[:], xT_o[:], wcat[:, D : 2 * D], start=False, stop=True)

                # elu(x) = max(x, exp(min(x,0)) - 1)
                t1 = wpool.tile([P, D], F32, tag="t1")
                out_t = wpool.tile([P, D], F32, tag="outt")
                nc.vector.tensor_scalar_min(t1[:], out_ps[:], 0.0)
                nc.scalar.activation(t1[:], t1[:], AF.Exp)
                nc.vector.scalar_tensor_tensor(
                    out_t[:], t1[:], -1.0, out_ps[:], AL.add, AL.max
                )
                nc.sync.dma_start(out_d[b0 : b0 + P], out_t[:])

            states = {}
            for it in range(n_tiles):
                states[it] = phase_a(it)
                if it >= 2:
                    phase_b(it - 2, *states.pop(it - 2))
            for it in (n_tiles - 2, n_tiles - 1):
                phase_b(it, *states.pop(it))

    nc.compile()
    return nc


_NC_CACHE = {}


def _get_nc(b_shard):
    if b_shard not in _NC_CACHE:
        _NC_CACHE[b_shard] = build_nc(b_shard)
    return _NC_CACHE[b_shard]


def _host_precompute(W_ally, W_opp, a_ally, a_opp, mask):
    v1a = W_ally @ a_ally[:D, 0]
    v2a = W_ally @ a_ally[D:, 0]
    v1o = W_opp @ a_opp[:D, 0]
    v2o = W_opp @ a_opp[D:, 0]
    vcat = np.ascontiguousarray(np.stack([v1a, v2a, v1o, v2o], axis=1).astype(BF16_NP))
    wcat = np.ascontiguousarray(np.concatenate([W_ally, W_opp], axis=1).astype(BF16_NP))
    # mask3[p, j, c] = 1 iff c == p % 32
    blk = (np.arange(32)[:, None] == np.arange(32)[None, :]).astype(BF16_NP)
    mask3 = np.ascontiguousarray(
        np.broadcast_to(np.tile(blk, (4, 1))[:, None, :], (P, NJ, 32)).reshape(
            P, NJ * 32
        )
    )
    ones = np.ones((P, 42), np.float32)
    ident = np.eye(P, dtype=BF16_NP)
    b = mask.shape[0]
    mneg = np.zeros((b, 42), np.float32)
    mneg[:, 1:21] = np.where(mask[:, 1 : 1 + NA], NEG_INF, 0.0)
    mneg[:, 22:42] = np.where(mask[:, 1 + NA :], NEG_INF, 0.0)
    return vcat, wcat, mask3, ones, ident, mneg


def kernel(h, W_ally, W_opp, a_ally, a_opp, mask, num_ally, num_opp):
    assert int(num_ally) == NA and int(num_opp) == NO
    h = np.asarray(h, dtype=np.float32)
    mask = np.asarray(mask)
    W_ally = np.asarray(W_ally, dtype=np.float32)
    W_opp = np.asarray(W_opp, dtype=np.float32)
    a_ally = np.asarray(a_ally, dtype=np.float32)
    a_opp = np.asarray(a_opp, dtype=np.float32)

    vcat, wcat, mask3, ones, ident, mneg = _host_precompute(W_ally, W_opp, a_ally, a_opp, mask)
    bfull = h.shape[0]
    h_bf = np.ascontiguousarray(h.reshape(bfull, NN * D).astype(BF16_NP))

    nc = _get_nc(B_SHARD)
    in_maps = []
    for c in range(N_CORES):
        s = slice(c * B_SHARD, (c + 1) * B_SHARD)
        in_maps.append(
            {
                "h": h_bf[s],
                "mneg": np.ascontiguousarray(mneg[s]),
                "vcat": vcat,
                "wcat": wcat,
                "mask3": mask3,
                "ones": ones,
                "ident": ident,
            }
        )
    res = run_bass_kernel_spmd(nc, in_maps, core_ids=list(range(N_CORES)))
    global LAST_RESULTS
    LAST_RESULTS = res
    return np.concatenate([res.results[c]["out"] for c in range(N_CORES)], axis=0)


LAST_RESULTS = None


# revision 15
# speedup vs baseline: 1.3284x; 1.0503x over previous
"""GroupGAT kernel for Trainium2 (Bass/Tile), 8-core data-parallel.

Math restructure (attention weights commute with @W):
    H = h @ W;  e[b,n] = lrelu(H_self.a1 + H[b,n].a2)
              = lrelu(h_self.(W a1) + h[b,n].(W a2))       <- dots in h-space
    ws = softmax(e) @ H = (softmax(e) @ h) @ W             <- weighted sum in h-space
    out = elu(H_ally[:,0] + ws_ally + ws_opp)
        = elu((h_self + hw_ally) @ W_ally + hw_opp @ W_opp)

v1-bf16: the DVE-chain structure of the original kernel, with
  - h in bf16, loaded as one flat 2D DMA per tile (contiguous descriptors);
  - hot DVE ops all-bf16 (2x_1P packed mode);
  - per-partition-scalar scaling via scalar_tensor_tensor (tensor_scalar
    with an AP scalar on bf16 operands hits a ~1.2us slow path);
  - diag builds / transposes / matmuls in bf16.
"""

import numpy as np
import ml_dtypes

import concourse.bass as bass
import concourse.bacc as bacc
import concourse.mybir as mybir
from concourse import tile
from concourse.bass_utils import run_bass_kernel_spmd

N_CORES = 8
B = 16384
NUM_NODE = 41
NA = 20  # num_ally
NO = 20  # num_opp
D = 128
B_SHARD = B // N_CORES
P = 128
NEG_INF = -1e9

F32 = mybir.dt.float32
BF16 = mybir.dt.bfloat16
AL = mybir.AluOpType
AF = mybir.ActivationFunctionType
BF16_NP = ml_dtypes.bfloat16


def build_nc(b_shard=B_SHARD):
    n_tiles = b_shard // P
    nc = bacc.Bacc("TRN2", target_bir_lowering=False, debug=False)

    h_d = nc.dram_tensor("h", [b_shard, NUM_NODE * D], BF16, kind="ExternalInput").ap()
    mneg_d = nc.dram_tensor("mneg", [b_shard, 42], F32, kind="ExternalInput").ap()
    vrep_d = nc.dram_tensor("vrep", [P, 4 * D], BF16, kind="ExternalInput").ap()
    wcat_d = nc.dram_tensor("wcat", [P, 2 * D], BF16, kind="ExternalInput").ap()
    ident_d = nc.dram_tensor("ident", [P, P], BF16, kind="ExternalInput").ap()
    zeros_d = nc.dram_tensor("zeros", [P, D], BF16, kind="ExternalInput").ap()
    out_d = nc.dram_tensor("out", [b_shard, D], F32, kind="ExternalOutput").ap()

    with tile.TileContext(nc) as tc:
        with (
            tc.tile_pool(name="const", bufs=1) as cpool,
            tc.tile_pool(name="hin", bufs=3) as hpool,
            tc.tile_pool(name="small", bufs=3) as spool,
            tc.tile_pool(name="work", bufs=3) as wpool,
            tc.tile_pool(name="psum", bufs=2, space=bass.MemorySpace.PSUM) as ppool,
        ):
            vrep = cpool.tile([P, 4 * D], BF16)
            wcat = cpool.tile([P, 2 * D], BF16)
            ident = cpool.tile([P, P], BF16)
            zerosD = cpool.tile([P, D], BF16)
            nc.sync.dma_start(vrep[:], vrep_d[:])
            nc.sync.dma_start(wcat[:], wcat_d[:])
            nc.sync.dma_start(ident[:], ident_d[:])
            nc.sync.dma_start(zerosD[:], zeros_d[:])
            v1a, v2a = vrep[:, 0:D], vrep[:, D : 2 * D]
            v1o, v2o = vrep[:, 2 * D : 3 * D], vrep[:, 3 * D : 4 * D]
            w_ally, w_opp = wcat[:, 0:D], wcat[:, D : 2 * D]

            for it in range(n_tiles):
                b0 = it * P
                h_t = hpool.tile([P, NUM_NODE * D], BF16)
                mneg_t = spool.tile([P, 42], F32, tag="mneg")
                nc.sync.dma_start(h_t[:], h_d[b0 : b0 + P])
                nc.sync.dma_start(mneg_t[:], mneg_d[b0 : b0 + P])

                def hnode(n):
                    return h_t[:, n * D : (n + 1) * D]

                prod = wpool.tile([P, D], BF16, tag="prod")
                prod_o = wpool.tile([P, D], BF16, tag="prod_o")
                s1 = spool.tile([P, 2], F32, tag="s1")
                e_a = spool.tile([P, 21], F32, tag="ea")
                e_o = spool.tile([P, 21], F32, tag="eo")

                # --- dots: s1x = h_self . v1x ; e_x[:,n] = h[n] . v2x
                nc.vector.scalar_tensor_tensor(
                    prod[:], hnode(0), 1.0, v1a, AL.mult, AL.mult, accum_out=s1[:, 0:1]
                )
                nc.vector.scalar_tensor_tensor(
                    prod_o[:], hnode(0), 1.0, v1o, AL.mult, AL.mult, accum_out=s1[:, 1:2]
                )
                for n in range(21):
                    nc.vector.scalar_tensor_tensor(
                        prod[:], hnode(n), 1.0, v2a,
                        AL.mult, AL.mult, accum_out=e_a[:, n : n + 1],
                    )
                for n in range(21):
                    src = 0 if n == 0 else NA + n
                    nc.vector.scalar_tensor_tensor(
                        prod_o[:], hnode(src), 1.0, v2o,
                        AL.mult, AL.mult, accum_out=e_o[:, n : n + 1],
                    )
                nc.vector.tensor_scalar_add(e_a[:], e_a[:], s1[:, 0:1])
                nc.vector.tensor_scalar_add(e_o[:], e_o[:], s1[:, 1:2])

                # --- leaky relu + additive mask
                nc.vector.scalar_tensor_tensor(
                    e_a[:], e_a[:], 0.2, e_a[:], AL.mult, AL.max
                )
                nc.vector.scalar_tensor_tensor(
                    e_o[:], e_o[:], 0.2, e_o[:], AL.mult, AL.max
                )
                nc.vector.tensor_add(e_a[:], e_a[:], mneg_t[:, 0:21])
                nc.vector.tensor_add(e_o[:], e_o[:], mneg_t[:, 21:42])

                # --- exp + fused denominator
                expe_a = spool.tile([P, 21], F32, tag="xpa")
                expe_o = spool.tile([P, 21], F32, tag="xpo")
                den = spool.tile([P, 2], F32, tag="den")
                rec = spool.tile([P, 2], F32, tag="rec")
                nc.scalar.activation(expe_a[:], e_a[:], AF.Exp, accum_out=den[:, 0:1])
                nc.scalar.activation(expe_o[:], e_o[:], AF.Exp, accum_out=den[:, 1:2])
                nc.vector.reciprocal(rec[:], den[:])

                # --- ally weighted sum via PE: diag(expe_n) @ h_n in PSUM
                diag_t = wpool.tile([P, 21, D], BF16, tag="diag")
                hwps_a = ppool.tile([P, D], F32, tag="hwa")
                for n in range(21):
                    nc.scalar.activation(
                        diag_t[:, n, :], ident[:], AF.Copy,
                        scale=expe_a[:, n : n + 1],
                    )
                for n in range(21):
                    nc.tensor.matmul(
                        hwps_a[:], diag_t[:, n, :], hnode(n),
                        start=(n == 0), stop=(n == 20),
                    )

                # --- opp weighted sum on DVE (bf16 chain)
                hw_o = wpool.tile([P, D], BF16, tag="hwo")
                nc.vector.scalar_tensor_tensor(
                    hw_o[:], hnode(0), expe_o[:, 0:1], zerosD[:], AL.mult, AL.add
                )
                for n in range(1, 21):
                    nc.vector.scalar_tensor_tensor(
                        hw_o[:], hnode(NA + n), expe_o[:, n : n + 1], hw_o[:],
                        AL.mult, AL.add,
                    )

                # --- x_a = h_self + hw_a/den_a ; x_o = hw_o/den_o  (bf16)
                x_a = wpool.tile([P, D], BF16, tag="xa")
                x_o = wpool.tile([P, D], BF16, tag="xo")
                nc.vector.scalar_tensor_tensor(
                    x_a[:], hwps_a[:], rec[:, 0:1], hnode(0), AL.mult, AL.add
                )
                nc.vector.scalar_tensor_tensor(
                    x_o[:], hw_o[:], rec[:, 1:2], zerosD[:], AL.mult, AL.add
                )

                # --- out = elu(x_a @ W_ally + x_o @ W_opp)
                tr = ppool.tile([P, 2, D], BF16, tag="tr")
                xT_a = wpool.tile([P, D], BF16, tag="xta")
                xT_o = wpool.tile([P, D], BF16, tag="xto")
                nc.tensor.transpose(tr[:, 0, :], x_a[:], ident[:])
                nc.tensor.transpose(tr[:, 1, :], x_o[:], ident[:])
                nc.scalar.copy(xT_a[:], tr[:, 0, :])
                nc.scalar.copy(xT_o[:], tr[:, 1, :])
                mm = ppool.tile([P, D], F32, tag="mm")
                nc.tensor.matmul(mm[:], xT_a[:], w_ally, start=True, stop=False)
                nc.tensor.matmul(mm[:], xT_o[:], w_opp, start=False, stop=True)

                # elu(x) = max(x, exp(min(x,0)) - 1)
                t1 = wpool.tile([P, D], F32, tag="t1")
                out_t = wpool.tile([P, D], F32, tag="outt")
                nc.vector.tensor_scalar_min(t1[:], mm[:], 0.0)
                nc.scalar.activation(t1[:], t1[:], AF.Exp)
                nc.vector.scalar_tensor_tensor(
                    out_t[:], t1[:], -1.0, mm[:], AL.add, AL.max
                )
                nc.sync.dma_start(out_d[b0 : b0 + P], out_t[:])

    nc.compile()
    return nc


_NC_CACHE = {}


def _get_nc(b_shard):
    if b_shard not in _NC_CACHE:
        _NC_CACHE[b_shard] = build_nc(b_shard)
    return _NC_CACHE[b_shard]


def _host_precompute(W_ally, W_opp, a_ally, a_opp, mask):
    v1a = (W_ally @ a_ally[:D, 0]).astype(np.float32)
    v2a = (W_ally @ a_ally[D:, 0]).astype(np.float32)
    v1o = (W_opp @ a_opp[:D, 0]).astype(np.float32)
    v2o = (W_opp @ a_opp[D:, 0]).astype(np.float32)
    vrep = np.concatenate(
        [np.broadcast_to(v[None, :], (P, D)) for v in (v1a, v2a, v1o, v2o)], axis=1
    ).astype(BF16_NP)
    vrep = np.ascontiguousarray(vrep)
    wcat = np.ascontiguousarray(
        np.concatenate([W_ally, W_opp], axis=1).astype(BF16_NP)
    )
    ident = np.eye(P, dtype=BF16_NP)
    zeros = np.zeros((P, D), dtype=BF16_NP)
    b = mask.shape[0]
    mneg = np.zeros((b, 42), np.float32)
    mneg[:, 1:21] = np.where(mask[:, 1 : 1 + NA], NEG_INF, 0.0)
    mneg[:, 22:42] = np.where(mask[:, 1 + NA :], NEG_INF, 0.0)
    return vrep, wcat, ident, zeros, mneg


def kernel(h, W_ally, W_opp, a_ally, a_opp, mask, num_ally, num_opp):
    assert int(num_ally) == NA and int(num_opp) == NO
    h = np.asarray(h, dtype=np.float32)
    mask = np.asarray(mask)
    W_ally = np.asarray(W_ally, dtype=np.float32)
    W_opp = np.asarray(W_opp, dtype=np.float32)
    a_ally = np.asarray(a_ally, dtype=np.float32)
    a_opp = np.asarray(a_opp, dtype=np.float32)

    vrep, wcat, ident, zeros, mneg = _host_precompute(
        W_ally, W_opp, a_ally, a_opp, mask
    )
    bfull = h.shape[0]
    h_bf = np.ascontiguousarray(h.reshape(bfull, NUM_NODE * D).astype(BF16_NP))

    nc = _get_nc(B_SHARD)
    in_maps = []
    for c in range(N_CORES):
        s = slice(c * B_SHARD, (c + 1) * B_SHARD)
        in_maps.append(
            {
                "h": h_bf[s],
                "mneg": np.ascontiguousarray(mneg[s]),
                "vrep": vrep,
                "wcat": wcat,
                "ident": ident,
                "zeros": zeros,
            }
        )
    res = run_bass_kernel_spmd(nc, in_maps, core_ids=list(range(N_CORES)))
    global LAST_RESULTS
    LAST_RESULTS = res
    return np.concatenate([res.results[c]["out"] for c in range(N_CORES)], axis=0)


LAST_RESULTS = None


# revision 16
# speedup vs baseline: 1.4443x; 1.0873x over previous
"""GroupGAT kernel for Trainium2 (Bass/Tile), 8-core data-parallel.

Math restructure (attention weights commute with @W):
    H = h @ W;  e[b,n] = lrelu(H_self.a1 + H[b,n].a2)
              = lrelu(h_self.(W a1) + h[b,n].(W a2))       <- dots in h-space
    ws = softmax(e) @ H = (softmax(e) @ h) @ W             <- weighted sum in h-space
    out = elu(H_ally[:,0] + ws_ally + ws_opp)
        = elu((h_self + hw_ally) @ W_ally + hw_opp @ W_opp)
so the big per-node matmuls collapse into per-node dot products (DVE) and
two [128,128] matmuls per batch tile (PE).
"""

import numpy as np

import concourse.bass as bass
import concourse.bacc as bacc
import concourse.mybir as mybir
from concourse import tile
from concourse.bass_utils import run_bass_kernel_spmd

N_CORES = 8
B = 16384
NUM_NODE = 41
NA = 20  # num_ally
NO = 20  # num_opp
D = 128
B_SHARD = B // N_CORES
P = 128  # partitions / batch tile
NEG_INF = -1e9

F32 = mybir.dt.float32
AL = mybir.AluOpType
AF = mybir.ActivationFunctionType


def build_nc(b_shard=B_SHARD, repeats=1):
    n_tiles = b_shard // P
    nc = bacc.Bacc("TRN2", target_bir_lowering=False, debug=False)

    h_d = nc.dram_tensor("h", [b_shard, NUM_NODE, D], F32, kind="ExternalInput").ap()
    mneg_d = nc.dram_tensor("mneg", [b_shard, 42], F32, kind="ExternalInput").ap()
    vrep_d = nc.dram_tensor("vrep", [P, 4 * D], F32, kind="ExternalInput").ap()
    wcat_d = nc.dram_tensor("wcat", [P, 2 * D], F32, kind="ExternalInput").ap()
    ident_d = nc.dram_tensor("ident", [P, P], F32, kind="ExternalInput").ap()
    out_d = nc.dram_tensor("out", [b_shard, D], F32, kind="ExternalOutput").ap()

    with tile.TileContext(nc) as tc:
        with (
            tc.tile_pool(name="const", bufs=1) as cpool,
            tc.tile_pool(name="hin", bufs=3) as hpool,
            tc.tile_pool(name="small", bufs=3) as spool,
            tc.tile_pool(name="work", bufs=3) as wpool,
            tc.tile_pool(name="psum", bufs=2, space=bass.MemorySpace.PSUM) as ppool,
        ):
            vrep = cpool.tile([P, 4 * D], F32)
            wcat = cpool.tile([P, 2 * D], F32)
            ident = cpool.tile([P, P], F32)
            nc.sync.dma_start(vrep[:], vrep_d[:])
            nc.sync.dma_start(wcat[:], wcat_d[:])
            nc.sync.dma_start(ident[:], ident_d[:])
            v1a, v2a = vrep[:, 0:D], vrep[:, D : 2 * D]
            v1o, v2o = vrep[:, 2 * D : 3 * D], vrep[:, 3 * D : 4 * D]
            w_ally, w_opp = wcat[:, 0:D], wcat[:, D : 2 * D]

            import contextlib

            rep_ctx = (
                tc.For_i(0, repeats, 1) if repeats > 1 else contextlib.nullcontext()
            )
            with rep_ctx:
                _body(nc, tc, n_tiles, h_d, mneg_d, out_d, hpool, spool, wpool,
                      ppool, v1a, v2a, v1o, v2o, w_ally, w_opp, ident)

    nc.compile()
    return nc


def _body(nc, tc, n_tiles, h_d, mneg_d, out_d, hpool, spool, wpool, ppool,
          v1a, v2a, v1o, v2o, w_ally, w_opp, ident):
    if True:  # keep indentation of the original loop body
            for it in range(n_tiles):
                b0 = it * P
                h_t = hpool.tile([P, NUM_NODE, D], F32)
                mneg_t = spool.tile([P, 42], F32)
                nc.sync.dma_start(h_t[:], h_d[b0 : b0 + P])
                nc.sync.dma_start(mneg_t[:], mneg_d[b0 : b0 + P])

                prod = wpool.tile([P, D], F32, tag="prod")
                s1 = spool.tile([P, 2], F32)
                e_a = spool.tile([P, 21], F32)
                e_o = spool.tile([P, 21], F32)

                # --- dots: s1x = h_self . v1x ; e_x[:,n] = h[n] . v2x  (+ s1x below)
                # (tensor_tensor_reduce faults the DVE on HW; scalar_tensor_tensor
                # with accum_out is the working multiply-reduce form)
                nc.vector.scalar_tensor_tensor(
                    prod[:], h_t[:, 0, :], 1.0, v1a, AL.mult, AL.mult, accum_out=s1[:, 0:1]
                )
                prod_o = wpool.tile([P, D], F32, tag="prod_o")
                nc.vector.scalar_tensor_tensor(
                    prod_o[:], h_t[:, 0, :], 1.0, v1o, AL.mult, AL.mult, accum_out=s1[:, 1:2]
                )
                for n in range(21):
                    nc.vector.scalar_tensor_tensor(
                        prod[:], h_t[:, n, :], 1.0, v2a,
                        AL.mult, AL.mult, accum_out=e_a[:, n : n + 1],
                    )
                for n in range(21):
                    src = 0 if n == 0 else NA + n
                    nc.vector.scalar_tensor_tensor(
                        prod_o[:], h_t[:, src, :], 1.0, v2o,
                        AL.mult, AL.mult, accum_out=e_o[:, n : n + 1],
                    )
                nc.vector.tensor_scalar_add(e_a[:], e_a[:], s1[:, 0:1])
                nc.vector.tensor_scalar_add(e_o[:], e_o[:], s1[:, 1:2])

                # --- leaky relu: lrelu(x) = max(0.2*x, x); then additive mask
                nc.vector.scalar_tensor_tensor(
                    e_a[:], e_a[:], 0.2, e_a[:], AL.mult, AL.max
                )
                nc.vector.scalar_tensor_tensor(
                    e_o[:], e_o[:], 0.2, e_o[:], AL.mult, AL.max
                )
                nc.vector.tensor_add(e_a[:], e_a[:], mneg_t[:, 0:21])
                nc.vector.tensor_add(e_o[:], e_o[:], mneg_t[:, 21:42])

                # --- exp (no max-sub needed; |e| <~ 20) + fused denominator
                expe_a = spool.tile([P, 21], F32)
                expe_o = spool.tile([P, 21], F32)
                den = spool.tile([P, 2], F32)
                rec = spool.tile([P, 2], F32)
                nc.scalar.activation(expe_a[:], e_a[:], AF.Exp, accum_out=den[:, 0:1])
                nc.scalar.activation(expe_o[:], e_o[:], AF.Exp, accum_out=den[:, 1:2])
                nc.vector.reciprocal(rec[:], den[:])

                # --- unnormalized weighted sums hw = sum_n expe[:,n] * h[:,n,:]
                # ally group via PE: diag(expe_n) @ h_n accumulated in PSUM
                # (ACT builds diag_n = ident * expe[:,n] with per-partition scale)
                diag_t = wpool.tile([P, 21, D], F32, tag="diag")
                hwps_a = ppool.tile([P, D], F32)
                for n in range(21):
                    nc.scalar.activation(
                        diag_t[:, n, :], ident[:], AF.Copy,
                        scale=expe_a[:, n : n + 1],
                    )
                for n in range(21):
                    nc.tensor.matmul(
                        hwps_a[:], diag_t[:, n, :], h_t[:, n, :],
                        start=(n == 0), stop=(n == 20),
                    )
                hw_o = wpool.tile([P, D], F32)
                nc.vector.tensor_scalar_mul(hw_o[:], h_t[:, 0, :], expe_o[:, 0:1])
                for n in range(1, 21):
                    nc.vector.scalar_tensor_tensor(
                        hw_o[:], h_t[:, NA + n, :], expe_o[:, n : n + 1], hw_o[:],
                        AL.mult, AL.add,
                    )

                # --- x_a = h_self + hw_a/den_a ; x_o = hw_o/den_o
                x_a = wpool.tile([P, D], F32)
                x_o = wpool.tile([P, D], F32)
                nc.vector.scalar_tensor_tensor(
                    x_a[:], hwps_a[:], rec[:, 0:1], h_t[:, 0, :], AL.mult, AL.add
                )
                nc.vector.tensor_scalar_mul(x_o[:], hw_o[:], rec[:, 1:2])

                # --- out = elu(x_a @ W_ally + x_o @ W_opp)
                tr_a = ppool.tile([P, D], F32)
                tr_o = ppool.tile([P, D], F32)
                xT_a = wpool.tile([P, D], F32)
                xT_o = wpool.tile([P, D], F32)
                nc.tensor.transpose(tr_a[:], x_a[:], ident[:])
                nc.tensor.transpose(tr_o[:], x_o[:], ident[:])
                nc.scalar.copy(xT_a[:], tr_a[:])
                nc.scalar.copy(xT_o[:], tr_o[:])
                mm = ppool.tile([P, D], F32)
                nc.tensor.matmul(mm[:], xT_a[:], w_ally, start=True, stop=False)
                nc.tensor.matmul(mm[:], xT_o[:], w_opp, start=False, stop=True)

                # elu(x) = max(x, exp(min(x,0)) - 1)
                t1 = wpool.tile([P, D], F32)
                out_t = wpool.tile([P, D], F32)
                nc.vector.tensor_scalar_min(t1[:], mm[:], 0.0)
                nc.scalar.activation(t1[:], t1[:], AF.Exp)
                nc.vector.scalar_tensor_tensor(
                    out_t[:], t1[:], -1.0, mm[:], AL.add, AL.max
                )
                nc.sync.dma_start(out_d[b0 : b0 + P], out_t[:])


_NC_CACHE = {}


def _get_nc(b_shard):
    if b_shard not in _NC_CACHE:
        _NC_CACHE[b_shard] = build_nc(b_shard)
    return _NC_CACHE[b_shard]


def _host_precompute(h, W_ally, W_opp, a_ally, a_opp, mask):
    b = h.shape[0]
    v1a = (W_ally @ a_ally[:D, 0]).astype(np.float32)
    v2a = (W_ally @ a_ally[D:, 0]).astype(np.float32)
    v1o = (W_opp @ a_opp[:D, 0]).astype(np.float32)
    v2o = (W_opp @ a_opp[D:, 0]).astype(np.float32)
    vrep = np.concatenate(
        [np.broadcast_to(v[None, :], (P, D)) for v in (v1a, v2a, v1o, v2o)], axis=1
    ).astype(np.float32)
    vrep = np.ascontiguousarray(vrep)
    wcat = np.ascontiguousarray(
        np.concatenate([W_ally, W_opp], axis=1).astype(np.float32)
    )
    ident = np.eye(P, dtype=np.float32)
    mneg = np.zeros((b, 42), np.float32)
    mneg[:, 1:21] = np.where(mask[:, 1 : 1 + NA], NEG_INF, 0.0)
    mneg[:, 22:42] = np.where(mask[:, 1 + NA :], NEG_INF, 0.0)
    return vrep, wcat, ident, mneg


def kernel(h, W_ally, W_opp, a_ally, a_opp, mask, num_ally, num_opp):
    assert int(num_ally) == NA and int(num_opp) == NO
    h = np.ascontiguousarray(np.asarray(h, dtype=np.float32))
    mask = np.asarray(mask)
    W_ally = np.asarray(W_ally, dtype=np.float32)
    W_opp = np.asarray(W_opp, dtype=np.float32)
    a_ally = np.asarray(a_ally, dtype=np.float32)
    a_opp = np.asarray(a_opp, dtype=np.float32)

    vrep, wcat, ident, mneg = _host_precompute(h, W_ally, W_opp, a_ally, a_opp, mask)

    nc = _get_nc(B_SHARD)
    in_maps = []
    for c in range(N_CORES):
        s = slice(c * B_SHARD, (c + 1) * B_SHARD)
        in_maps.append(
            {
                "h": h[s],
                "mneg": np.ascontiguousarray(mneg[s]),
                "vrep": vrep,
                "wcat": wcat,
                "ident": ident,
            }
        )
    res = run_bass_kernel_spmd(nc, in_maps, core_ids=list(range(N_CORES)))
    global LAST_RESULTS
    LAST_RESULTS = res
    return np.concatenate([res.results[c]["out"] for c in range(N_CORES)], axis=0)


LAST_RESULTS = None

